# revision 4
# baseline (speedup 1.0000x reference)
"""3-layer GCN (GCNConv x3, tanh between) on 8 Trainium2 NeuronCores.

Strategy (v2 — "SpMM-first" restructure of the node-range-sharded scheme):
  - GCN aggregation commutes with the dense transform (both linear), so
    layer 1 aggregates the *input features* x directly: each core holds
    the full bf16 copy of x in DRAM as a gather table, so layer-1 message
    gathering starts at t=0 with no AllGather and 256-wide (not 512-wide)
    rows. The aggregated block is then densified locally:
        z1 = tanh(aggT_x^T @ W1 + b1).
  - Layer 2 is also SpMM-first on the AllGathered z1 table. The chunk
    matmuls are emitted transposed (aggT_k = G_k^T @ S, [fin_chunk, dst])
    so the aggregate lands feature-major and feeds the dense matmul's
    stationary operand directly — no transposes for z1/z2-in.
  - Layer 3 is dense-first (h3 = z2 @ W3 is 256-wide, halving both its
    AllGather and its gather traffic); z2 is transposed on the
    TensorEngine (bf16, 1 cyc/row) to feature-major for that matmul.
  - Edges (+ one self-edge per node, weight dinv^2) are bucketed per
    128-dst-node block and *deduplicated by src* within the block
    (~10% fewer gathered rows and chunks); the per-chunk S matrix
    [row, dst_local] accumulates duplicate edge weights. One S table
    serves all three layers. Gathers use the GPSIMD dma_gather extended
    instruction batched 8 chunks / 1024 rows per instruction.
  - All matmuls are bf16 (1 cycle/row on the PE) accumulating fp32 in
    PSUM; biases enter PSUM as a rank-1 ones^T @ b matmul.

Numerics: messages, aggregates, and weights are bf16; accumulation is
fp32. End-to-end relative L2 error vs the fp32 reference is ~5e-3.
Host preprocessing touches only edge_index (sorting/bincount/unique),
the degree-derived edge weights, and bf16 casts of x/W/b.
"""
import sys

if "/opt/trn_rl_repo" not in sys.path:
    sys.path.insert(0, "/opt/trn_rl_repo")

from contextlib import ExitStack

import ml_dtypes
import numpy as np

import concourse.bass as bass
import concourse.bacc as bacc
import concourse.mybir as mybir
import concourse.tile as tile
from concourse.bass_utils import run_bass_kernel_spmd
from concourse.masks import make_identity

P = 128
N_CORES = 8
N_NODES = 10000
SHARD = N_NODES // N_CORES          # 1250
N_BLOCKS = (SHARD + P - 1) // P     # 10 (9 full + one 98-row block)
IN_DIM, HID_DIM, OUT_DIM = 256, 512, 256
SA, SB = 768, SHARD - 768           # split-AllGather half sizes
GK = 8                              # gather chunks per dma_gather instr

_DT = mybir.dt.float32
_DTG = mybir.dt.bfloat16

_BF = ml_dtypes.bfloat16


# ----------------------------------------------------------------------------
# Host-side edge preprocessing
# ----------------------------------------------------------------------------

def _preprocess(edge_index: np.ndarray):
    """Bucket edges by dst block, dedup srcs per block, build S + gidx.

    Returns (schedule, gidx1_pc, gidxA_pc, s_pc):
      schedule[b] : chunk count for dst-block b (shared by all cores)
      gidx1_pc    : [P, C*8] int16 per core, plain node-id gather indices
                    (for the replicated x table)
      gidxA_pc    : [P, C*8] int16 per core, split-AllGather-layout indices
                    (for the hf1/hf3 tables)
      s_pc        : [P, C*P] bf16 per core, chunk-major S (dedup-accumulated
                    edge weights, S[row, dst_local])
    """
    src = np.asarray(edge_index[0], dtype=np.int64)
    dst = np.asarray(edge_index[1], dtype=np.int64)

    deg = (np.bincount(dst, minlength=N_NODES) + 1.0).astype(np.float32)
    dinv = (1.0 / np.sqrt(deg.astype(np.float64))).astype(np.float32)

    all_src = np.concatenate([src, np.arange(N_NODES, dtype=np.int64)])
    all_dst = np.concatenate([dst, np.arange(N_NODES, dtype=np.int64)])
    all_w = np.concatenate([dinv[src] * dinv[dst], dinv * dinv]).astype(np.float32)

    per_core = []
    nuniq = np.zeros((N_CORES, N_BLOCKS), dtype=np.int64)
    for c in range(N_CORES):
        lo = c * SHARD
        mask = (all_dst >= lo) & (all_dst < lo + SHARD)
        csrc, cdst, cw = all_src[mask], all_dst[mask] - lo, all_w[mask]
        blocks = []
        for b in range(N_BLOCKS):
            bm = (cdst >= b * P) & (cdst < (b + 1) * P)
            bsrc, bdst, bw = csrc[bm], cdst[bm] - b * P, cw[bm]
            uniq, inv = np.unique(bsrc, return_inverse=True)
            blocks.append((uniq, inv, bdst, bw))
            nuniq[c, b] = len(uniq)
        per_core.append(blocks)

    schedule = [int(x) for x in ((nuniq.max(axis=0) + P - 1) // P)]
    C = sum(schedule)
    cbases = np.concatenate([[0], np.cumsum(schedule)])

    gidx1_pc, gidxA_pc, s_pc = [], [], []
    for c in range(N_CORES):
        flat = np.zeros(C * P, dtype=np.int64)
        S = np.zeros((C * P, P), dtype=np.float32)
        for b in range(N_BLOCKS):
            uniq, inv, bdst, bw = per_core[c][b]
            r0 = cbases[b] * P
            flat[r0:r0 + len(uniq)] = uniq
            np.add.at(S, (r0 + inv, bdst), bw)
        # Split-AllGather hfull layout:
        # node n = r*SHARD + q -> r*SA + q             (q < SA,  first half)
        #                      -> 8*SA + r*SB + (q-SA) (q >= SA, second half)
        r_, q_ = flat // SHARD, flat % SHARD
        flat_ag = np.where(q_ < SA, r_ * SA + q_, 8 * SA + r_ * SB + (q_ - SA))

        # dma_gather int16 index layout: flat index i -> [i % 16, i // 16],
        # replicated across the 8 GPSIMD-core partition groups.
        def wrap(f):
            w = f.astype(np.int16).reshape(C * P // 16, 16).T
            return np.tile(w, (8, 1)).copy()

        gidx1_pc.append(wrap(flat))
        gidxA_pc.append(wrap(flat_ag))
        S2 = S.reshape(-1, P, P).transpose(1, 0, 2).reshape(P, -1)
        s_pc.append(np.ascontiguousarray(S2).astype(_BF))
    return schedule, gidx1_pc, gidxA_pc, s_pc


# ----------------------------------------------------------------------------
# Device kernel
# ----------------------------------------------------------------------------

def _build(schedule, nrep=1):
    C = sum(schedule)
    nc = bacc.Bacc("TRN2", num_devices=N_CORES)

    xg = nc.dram_tensor("xg", [N_NODES, IN_DIM], _DTG, kind="ExternalInput")
    W1 = nc.dram_tensor("W1", [IN_DIM, HID_DIM], _DTG, kind="ExternalInput")
    W2 = nc.dram_tensor("W2", [HID_DIM, HID_DIM], _DTG, kind="ExternalInput")
    W3 = nc.dram_tensor("W3", [HID_DIM, OUT_DIM], _DTG, kind="ExternalInput")
    b1 = nc.dram_tensor("b1", [1, HID_DIM], _DTG, kind="ExternalInput")
    b2 = nc.dram_tensor("b2", [1, HID_DIM], _DTG, kind="ExternalInput")
    b3 = nc.dram_tensor("b3", [1, OUT_DIM], _DTG, kind="ExternalInput")
    gidx1 = nc.dram_tensor("gidx1", [P, C * 8], mybir.dt.int16, kind="ExternalInput")
    gidxA = nc.dram_tensor("gidxA", [P, C * 8], mybir.dt.int16, kind="ExternalInput")
    S = nc.dram_tensor("S", [P, C * P], _DTG, kind="ExternalInput")
    out = nc.dram_tensor("out", [SHARD, OUT_DIM], _DT, kind="ExternalOutput")

    hs1 = nc.dram_tensor("hs1", [SHARD, HID_DIM], _DTG)
    hs3 = nc.dram_tensor("hs3", [SHARD, OUT_DIM], _DTG)
    hf1 = nc.dram_tensor("hf1", [N_NODES, HID_DIM], _DTG, addr_space="Shared")
    hf3 = nc.dram_tensor("hf3", [N_NODES, OUT_DIM], _DTG, addr_space="Shared")

    rg = [list(range(N_CORES))]

    cbases = [0]
    for b in range(N_BLOCKS):
        cbases.append(cbases[-1] + schedule[b])

    with tile.TileContext(nc) as tc, ExitStack() as ctx:
        const = ctx.enter_context(tc.tile_pool(name="const", bufs=1))
        gp = ctx.enter_context(tc.tile_pool(name="gather", bufs=4))
        ab = ctx.enter_context(tc.tile_pool(name="aggt", bufs=2))
        hp = ctx.enter_context(tc.tile_pool(name="hb", bufs=3))
        op = ctx.enter_context(tc.tile_pool(name="ob", bufs=2))
        psa = ctx.enter_context(tc.tile_pool(name="psa", bufs=2, space="PSUM"))
        psd = ctx.enter_context(tc.tile_pool(name="psd", bufs=2, space="PSUM"))
        pst = ctx.enter_context(tc.tile_pool(name="pst", bufs=2, space="PSUM"))

        ident = const.tile([P, P], _DTG)
        make_identity(nc, ident[:])
        onesb = const.tile([1, P], _DTG)
        nc.vector.memset(onesb[:], 1.0)

        # gather-critical loads first on the SP queue: gidx1, then S slices
        # (emitted per-block inside the L1 loop)
        gidx1_t = const.tile([P, C * 8], mybir.dt.int16)
        nc.sync.dma_start(out=gidx1_t[:], in_=gidx1[:])
        s_all = const.tile([P, C * P], _DTG)

        # weights / biases / gidxA on the Activation queue (not gather-
        # critical; keeps the SP queue free for S slices)
        gidxA_t = const.tile([P, C * 8], mybir.dt.int16)
        nc.scalar.dma_start(out=gidxA_t[:], in_=gidxA[:])

        w_tiles, b_tiles = [], []
        for W, b, fin, fout in [(W1, b1, IN_DIM, HID_DIM),
                                (W2, b2, HID_DIM, HID_DIM),
                                (W3, b3, HID_DIM, OUT_DIM)]:
            nk = fin // P
            wt = const.tile([P, nk * fout], _DTG, tag=f"w{fin}x{fout}")
            for k in range(nk):
                nc.scalar.dma_start(
                    out=wt[:].rearrange("p (k f) -> p k f", k=nk)[:, k:k + 1, :],
                    in_=W[:].rearrange("(k p) f -> p k f", p=P)[:, k:k + 1, :])
            bt = const.tile([1, fout], _DTG, tag=f"b{fout}")
            nc.scalar.dma_start(out=bt[:], in_=b[:])
            w_tiles.append(wt)
            b_tiles.append(bt)

        z2T = const.tile([P, (HID_DIM // P) * SHARD], _DTG)

        def gathers(table, gidx_t, fin, d):
            """Issue batched gathers for dst-block d; yield (c, gt, off)."""
            nchunks, cbase = schedule[d], cbases[d]
            tiles = []
            for g0 in range(0, nchunks, GK):
                g1 = min(g0 + GK, nchunks)
                n_sub = g1 - g0
                gt = gp.tile([P, GK * HID_DIM], _DTG, tag="g")
                nc.gpsimd.dma_gather(
                    out_ap=gt[:, :n_sub * fin].rearrange(
                        "p (c f) -> p c f", c=n_sub),
                    in_ap=table[:],
                    idxs_ap=gidx_t[:, (cbase + g0) * 8: (cbase + g1) * 8],
                    num_idxs=n_sub * P,
                    num_idxs_reg=n_sub * P,
                    elem_size=fin,
                )
                tiles.append((g0, g1, gt))
            return tiles

        def spmm_aggT(li, d):
            """Aggregate dst-block d transposed: aggT[k] = G_k^T @ S (bf16).

            k is the outer loop: PSUM accumulation-group starts zero the
            whole 2 KiB zero region, so groups must be strictly sequential
            (start..stop before the next start) within a region.
            """
            fin = IN_DIM if li == 0 else HID_DIM
            nk = fin // P
            table = xg if li == 0 else hf1
            gidx_t = gidx1_t if li == 0 else gidxA_t
            nchunks, cbase = schedule[d], cbases[d]
            tiles = gathers(table, gidx_t, fin, d)
            at = ab.tile([P, 4 * P], _DTG, tag="at")
            for k in range(nk):
                ps = psa.tile([P, P], _DT, tag="psa")
                for g0, g1, gt in tiles:
                    for c in range(g0, g1):
                        nc.tensor.matmul(
                            ps[:],
                            lhsT=gt[:, (c - g0) * fin + k * P:
                                    (c - g0) * fin + (k + 1) * P],
                            rhs=s_all[:, (cbase + c) * P:(cbase + c + 1) * P],
                            start=(c == 0),
                            stop=(c == nchunks - 1),
                        )
                nc.vector.tensor_copy(at[:, k * P:(k + 1) * P], ps[:])
            return at

        def dense(li, d, at):
            """z_{li+1} block d = tanh(aggT^T @ W + b), bf16 node-major."""
            fin = IN_DIM if li == 0 else HID_DIM
            fout = HID_DIM
            nk = fin // P
            nd = min(P, SHARD - d * P)
            wt, bt = w_tiles[li], b_tiles[li]
            ps = psd.tile([P, HID_DIM], _DT, tag="psd")
            for k in range(nk):
                nc.tensor.matmul(
                    ps[:nd, :fout],
                    lhsT=at[:, k * P:k * P + nd],
                    rhs=wt[:, k * fout:(k + 1) * fout],
                    start=(k == 0),
                    stop=False,
                )
            nc.tensor.matmul(
                ps[:nd, :fout], lhsT=onesb[:, :nd], rhs=bt[:],
                start=False, stop=True,
            )
            hbt = hp.tile([P, HID_DIM], _DTG, tag="hb")
            nc.scalar.activation(
                hbt[:nd, :fout], ps[:nd, :fout],
                mybir.ActivationFunctionType.Tanh)
            return hbt

        def ag_half(hs_t, hf_t, half):
            if half == 0:
                ins_, outs_ = hs_t[:SA, :], hf_t[:N_CORES * SA, :]
            else:
                ins_, outs_ = hs_t[SA:, :], hf_t[N_CORES * SA:, :]
            nc.gpsimd.collective_compute(
                "AllGather",
                mybir.AluOpType.bypass,
                replica_groups=rg,
                ins=[ins_],
                outs=[outs_],
            )

        # ---- Layer 1: SpMM(x) -> dense W1 -> tanh -> hs1/AG ----
        for d in range(N_BLOCKS):
            nc.sync.dma_start(
                out=s_all[:, cbases[d] * P: cbases[d + 1] * P],
                in_=S[:, cbases[d] * P: cbases[d + 1] * P])
            nd = min(P, SHARD - d * P)
            at = spmm_aggT(0, d)
            hbt = dense(0, d, at)
            nc.sync.dma_start(out=hs1[d * P: d * P + nd, :], in_=hbt[:nd, :])
            if d == 5:
                ag_half(hs1, hf1, 0)
        ag_half(hs1, hf1, 1)

        # ---- Layer 2: SpMM(z1) -> dense W2 -> tanh -> z2T; L3 dense ----
        for d in range(N_BLOCKS):
            nd = min(P, SHARD - d * P)
            at = spmm_aggT(1, d)
            hbt = dense(1, d, at)
            for k in range(HID_DIM // P):
                pt = pst.tile([P, P], _DTG, tag="pst")
                nc.tensor.transpose(
                    out=pt[:, :nd],
                    in_=hbt[:nd, k * P:(k + 1) * P],
                    identity=ident[:nd, :nd],
                )
                nc.vector.tensor_copy(
                    z2T[:, k * SHARD + d * P: k * SHARD + d * P + nd],
                    pt[:, :nd],
                )
            ps3 = psd.tile([P, HID_DIM], _DT, tag="psd")
            for k in range(HID_DIM // P):
                nc.tensor.matmul(
                    ps3[:nd, :OUT_DIM],
                    lhsT=z2T[:, k * SHARD + d * P: k * SHARD + d * P + nd],
                    rhs=w_tiles[2][:, k * OUT_DIM:(k + 1) * OUT_DIM],
                    start=(k == 0),
                    stop=(k == HID_DIM // P - 1),
                )
            hb3 = hp.tile([P, HID_DIM], _DTG, tag="hb")
            nc.scalar.activation(
                hb3[:nd, :OUT_DIM], ps3[:nd, :OUT_DIM],
                mybir.ActivationFunctionType.Copy)
            nc.sync.dma_start(
                out=hs3[d * P: d * P + nd, :], in_=hb3[:nd, :OUT_DIM])
            if d == 5:
                ag_half(hs3, hf3, 0)
        ag_half(hs3, hf3, 1)

        # ---- Layer 3: SpMM(h3) + b3 -> out ----
        for d in range(N_BLOCKS):
            nd = min(P, SHARD - d * P)
            nchunks, cbase = schedule[d], cbases[d]
            ps = psd.tile([P, HID_DIM], _DT, tag="psd")
            for g0, g1, gt in gathers(hf3, gidxA_t, OUT_DIM, d):
                for c in range(g0, g1):
                    nc.tensor.matmul(
                        ps[:, :OUT_DIM],
                        lhsT=s_all[:, (cbase + c) * P:(cbase + c + 1) * P],
                        rhs=gt[:, (c - g0) * OUT_DIM:(c - g0 + 1) * OUT_DIM],
                        start=(c == 0),
                        stop=False,
                    )
            nc.tensor.matmul(
                ps[:, :OUT_DIM], lhsT=onesb[:], rhs=b_tiles[2][:],
                start=False, stop=True,
            )
            ob = op.tile([P, OUT_DIM], _DT, tag="ob")
            nc.vector.tensor_copy(ob[:nd], ps[:nd, :OUT_DIM])
            nc.sync.dma_start(out=out[d * P: d * P + nd, :], in_=ob[:nd])

    nc.compile()
    return nc


_CACHE = {}


def _get_kernel(schedule, nrep=1):
    key = (tuple(schedule), nrep)
    if key not in _CACHE:
        _CACHE[key] = _build(schedule, nrep)
    return _CACHE[key]


# ----------------------------------------------------------------------------
# Entry point
# ----------------------------------------------------------------------------

def kernel(x, W1, b1, W2, b2, W3, b3, edge_index, _trace=False, _trace_kwargs=None):
    x = np.asarray(x, dtype=np.float32)
    Ws = [np.ascontiguousarray(np.asarray(w, dtype=np.float32).astype(_BF))
          for w in (W1, W2, W3)]
    bs = [np.ascontiguousarray(
        np.asarray(b, dtype=np.float32).reshape(1, -1).astype(_BF))
        for b in (b1, b2, b3)]
    edge_index = np.asarray(edge_index)

    xg = np.ascontiguousarray(x.astype(_BF))
    schedule, gidx1_pc, gidxA_pc, s_pc = _preprocess(edge_index)
    nc = _get_kernel(schedule)

    in_maps = []
    for c in range(N_CORES):
        in_maps.append({
            "xg": xg,
            "W1": Ws[0], "W2": Ws[1], "W3": Ws[2],
            "b1": bs[0], "b2": bs[1], "b3": bs[2],
            "gidx1": gidx1_pc[c],
            "gidxA": gidxA_pc[c],
            "S": s_pc[c],
        })

    kwargs = {}
    if _trace:
        kwargs = {"trace": True, "trace_kwargs": _trace_kwargs or {}}
    try:
        res = run_bass_kernel_spmd(
            nc, in_maps, core_ids=list(range(N_CORES)), **kwargs)
    except Exception:
        # transient axon/device errors (e.g. NRT_EXEC_UNIT_UNRECOVERABLE on a
        # cold worker) clear on re-execution; retry once
        res = run_bass_kernel_spmd(
            nc, in_maps, core_ids=list(range(N_CORES)), **kwargs)
    out = np.concatenate([res.results[c]["out"] for c in range(N_CORES)], axis=0)
    if _trace:
        return out, res
    return out
